# revision 17
# baseline (speedup 1.0000x reference)
"""3-level db4 wavelet low/high split for (32, 64, 16384) fp32 on 8 TRN2 NeuronCores.

Math: the reference computes wavedec (3-level db4, symmetric padding) then two
waverecs: `low` (details zeroed) and `high` (approximation zeroed).  Wavelets
give perfect reconstruction, so low + high == x and only the lowpass path is
needed: low = G @ (H @ x_row) with H (2054 x 16384) the composite 3-level
lowpass analysis operator (symmetric extension folded in) and G (16384 x 2054)
the lowpass synthesis operator; high = x - low on-chip.

Layout/sharding: the signal axis L is sharded across the 8 cores (2048
positions each + 128-position halo).  The host uploads x TRANSPOSED into
"sig" layout [pos, rows] as bf16, so every on-device matmul streams the 2048
fused batch*feature rows as the moving operand and NO on-device transposes
are needed; outputs are written back in sig layout as bf16 and the host
re-transposes while unsharding.  Both operator stages are banded: per core
only 20 (stage 1) + 18 (stage 2) distinct 128x128 weight tiles are nonzero.

Device pipeline per core:
 - 18 x-tile DMAs [128 pos, 2048 rows] bf16 (4 KiB/partition lines)
 - stage 1 (PE, bf16): a3[t] += W1(t,pb)^T @ x[pb], accumulated in PSUM
   (4 banks of [128, 512] per a-tile), ACT-copied to SBUF bf16
 - stage 2 (PE, bf16): low[o] += W2(o,t)^T @ a3[t] in PSUM
 - ACT: low PSUM -> SBUF bf16; DVE: high = x - low(PSUM) -> bf16
 - per-o DMAs of low/high [128, 2048] bf16 back to HBM
"""

import numpy as np
import scipy.sparse as sp
import ml_dtypes

import concourse.bacc as bacc
import concourse.tile as tile
from concourse import mybir
from concourse.bass_utils import run_bass_kernel_spmd

F32 = mybir.dt.float32
BF16 = mybir.dt.bfloat16
BF16_NP = ml_dtypes.bfloat16

DEC_LO = np.array([-0.010597401785069032, 0.032883011666982945, 0.030841381835986965,
                   -0.18703481171888114, -0.02798376941698385, 0.6308807679295904,
                   0.7148465705525415, 0.23037781330885523], dtype=np.float64)
REC_LO = DEC_LO[::-1].copy()
F = 8
N_CORES = 8
L = 16384
ROWS = 2048          # fused B*F rows
S = L // N_CORES     # 2048 positions per core
NPB = 18             # local x tiles (128-position halo each side)
NT = 3               # local a3 tiles (384-value a window)
NO = 16              # local output tiles
A_OFF = -64          # a-window start, relative to 256*c
X_OFF = -128         # x-window start, relative to 2048*c
NCHUNK = 4           # 2048 rows in 4 PSUM-bank chunks of 512


def _symidx(n):
    idx = np.concatenate([np.arange(6, -1, -1), np.arange(n), np.arange(n - 1, n - 8, -1)])
    return idx[1:]


def _dwt_lo_mat(n):
    ext_idx = _symidx(n)
    lout = (n + 13 - F) // 2 + 1
    filt = DEC_LO[::-1]
    rows = np.repeat(np.arange(lout), F)
    cols = ext_idx[(2 * np.arange(lout)[:, None] + np.arange(F)[None, :]).ravel()]
    vals = np.tile(filt, lout)
    return sp.coo_matrix((vals, (rows, cols)), shape=(lout, n)).tocsr()


def _idwt_lo_mat(n):
    lout = 2 * n + 1 - F + 1
    filt = REC_LO[::-1]
    rows, cols, vals = [], [], []
    i = np.arange(lout)
    for k in range(F):
        pos = i + k
        m = (pos % 2 == 1)
        rows.append(i[m])
        cols.append((pos[m] - 1) // 2)
        vals.append(np.full(int(m.sum()), filt[k]))
    return sp.coo_matrix(
        (np.concatenate(vals), (np.concatenate(rows), np.concatenate(cols))),
        shape=(lout, n)).tocsr()


def _build_H_G(L, level=3):
    H = sp.identity(L, format="csr")
    lens = []
    n = L
    for _ in range(level):
        lens.append(n)
        D = _dwt_lo_mat(n)
        H = D @ H
        n = D.shape[0]
    G = sp.identity(n, format="csr")
    a_len = n
    for ln in lens[::-1]:
        d_len = (ln + F - 1) // 2
        if a_len == d_len + 1:
            G = sp.identity(a_len, format="csr")[:-1] @ G
            a_len -= 1
        U = _idwt_lo_mat(a_len)
        G = U @ G
        a_len = U.shape[0]
    return H, G


def _slice_pad(M, r0, r1, c0, c1):
    out = np.zeros((r1 - r0, c1 - c0), np.float32)
    rr0, rr1 = max(r0, 0), min(r1, M.shape[0])
    cc0, cc1 = max(c0, 0), min(c1, M.shape[1])
    if rr0 < rr1 and cc0 < cc1:
        out[rr0 - r0:rr1 - r0, cc0 - c0:cc1 - c0] = M[rr0:rr1, cc0:cc1]
    return out


def _build_plan():
    H, G = _build_H_G(L)
    HT = np.asarray(H.T.todense(), np.float32)   # [L, na]
    GT = np.asarray(G.T.todense(), np.float32)   # [na, L]

    s1_pairs, s2_pairs = set(), set()
    w1, w2 = {}, {}
    for c in range(N_CORES):
        xbase = 2048 * c + X_OFF
        abase = 256 * c + A_OFF
        for t in range(NT):
            a0 = abase + 128 * t
            for pb in range(NPB):
                p0 = xbase + 128 * pb
                tl = _slice_pad(HT, p0, p0 + 128, a0, a0 + 128)
                if np.any(tl):
                    s1_pairs.add((t, pb))
                    w1[(c, t, pb)] = tl
        for o in range(NO):
            i0 = 2048 * c + 128 * o
            for t in range(NT):
                a0 = abase + 128 * t
                tl = _slice_pad(GT, a0, a0 + 128, i0, i0 + 128)
                if np.any(tl):
                    s2_pairs.add((o, t))
                    w2[(c, o, t)] = tl
    s1_pairs = sorted(s1_pairs)
    s2_pairs = sorted(s2_pairs)

    # weight blobs per core, one 128x128 slot per pair (zeros where the core
    # has no tile); stage-1 slots keyed by (t, pb), stage-2 by (o, t)
    w1_blob = np.zeros((N_CORES, 128, 128 * len(s1_pairs)), BF16_NP)
    w2_blob = np.zeros((N_CORES, 128, 128 * len(s2_pairs)), BF16_NP)
    for c in range(N_CORES):
        for i, (t, pb) in enumerate(s1_pairs):
            tl = w1.get((c, t, pb))
            if tl is not None:
                w1_blob[c, :, 128 * i:128 * i + 128] = tl.astype(BF16_NP)
        for i, (o, t) in enumerate(s2_pairs):
            tl = w2.get((c, o, t))
            if tl is not None:
                w2_blob[c, :, 128 * i:128 * i + 128] = tl.astype(BF16_NP)

    s1groups = [[] for _ in range(NT)]          # t -> [(pb, slot)]
    for i, (t, pb) in enumerate(s1_pairs):
        s1groups[t].append((pb, i))
    s2groups = [[] for _ in range(NO)]          # o -> [(t, slot)]
    for i, (o, t) in enumerate(s2_pairs):
        s2groups[o].append((t, i))
    return dict(w1=w1_blob, w2=w2_blob, s1groups=s1groups, s2groups=s2groups,
                n1=len(s1_pairs), n2=len(s2_pairs))


def _build_program(plan):
    nc = bacc.Bacc("TRN2", target_bir_lowering=False, debug=False)
    x_d = nc.dram_tensor("x", [NPB * 128, ROWS], BF16, kind="ExternalInput").ap()
    w1_d = nc.dram_tensor("w1", [128, 128 * plan["n1"]], BF16, kind="ExternalInput").ap()
    w2_d = nc.dram_tensor("w2", [128, 128 * plan["n2"]], BF16, kind="ExternalInput").ap()
    # low and high interleaved per position row -> 8 KiB DMA partition lines
    out_d = nc.dram_tensor("out", [S, 2 * ROWS], BF16, kind="ExternalOutput").ap()

    CH = ROWS // NCHUNK  # 512

    with tile.TileContext(nc) as tc:
        with tc.tile_pool(name="sbw", bufs=1) as sbw, \
             tc.tile_pool(name="sbx", bufs=1) as sbx, \
             tc.tile_pool(name="sba3", bufs=1) as sba3, \
             tc.tile_pool(name="sbo", bufs=6) as sbo, \
             tc.tile_pool(name="psa", bufs=1, space="PSUM") as psa, \
             tc.tile_pool(name="ps2", bufs=4, space="PSUM") as ps2:

            # weights via scalar/vector triggers (idle at program head); the
            # first w1 quarter (earliest-needed slots) partition-split for
            # low arrival latency
            w1t = sbw.tile([128, 128 * plan["n1"]], BF16, tag="w1t")
            q = (plan["n1"] * 128) // 4
            nc.scalar.dma_start(w1t[0:64, 0:q], w1_d[0:64, 0:q])
            nc.scalar.dma_start(w1t[64:128, 0:q], w1_d[64:128, 0:q])
            nc.scalar.dma_start(w1t[:, q:2 * q], w1_d[:, q:2 * q])
            nc.scalar.dma_start(w1t[:, 2 * q:3 * q], w1_d[:, 2 * q:3 * q])
            nc.scalar.dma_start(w1t[:, 3 * q:4 * q], w1_d[:, 3 * q:4 * q])
            w2t = sbw.tile([128, 128 * plan["n2"]], BF16, tag="w2t")
            q2 = (plan["n2"] * 128) // 2
            nc.scalar.dma_start(w2t[:, 0:q2], w2_d[:, 0:q2])
            nc.scalar.dma_start(w2t[:, q2:2 * q2], w2_d[:, q2:2 * q2])

            # x tiles: every tile partition-split across two trigger engines
            # (sync + gpsimd) so arrivals stay dense for the PE.  The halo
            # tiles only need a sliver of real data (stage-2 never reads the
            # a3 values their dead positions feed) — load the sliver and
            # memset the rest so no NaN garbage enters the matmuls.
            xt = [sbx.tile([128, ROWS], BF16, tag=f"x{pb}", name=f"x{pb}")
                  for pb in range(NPB)]
            nc.vector.memset(xt[0][0:104, :], 0)
            nc.vector.memset(xt[NPB - 1][64:128, :], 0)
            nc.sync.dma_start(xt[0][104:128, :], x_d[104:128, :])
            p17 = 128 * (NPB - 1)
            nc.gpsimd.dma_start(xt[NPB - 1][0:64, :], x_d[p17:p17 + 64, :])
            for pb in range(1, NPB - 1):
                nc.sync.dma_start(xt[pb][0:64, :], x_d[128 * pb:128 * pb + 64, :])
                nc.gpsimd.dma_start(
                    xt[pb][64:128, :], x_d[128 * pb + 64:128 * pb + 128, :])

            # output tiles o grouped by the last a3 tile they need
            o_after_t = [[] for _ in range(NT)]
            for o in range(NO):
                o_after_t[max(t for t, _ in plan["s2groups"][o])].append(o)

            a3 = [None] * NT
            for t in range(NT):
                ents = plan["s1groups"][t]
                a3t = sba3.tile([128, ROWS], BF16, tag=f"a3_{t}", name=f"a3_{t}")
                # pair-outer so each x tile is consumed the moment it lands;
                # the four chunk accumulation groups interleave in PSUM
                pa = [psa.tile([128, CH], F32, tag=f"pa{k}", name=f"pa{k}")
                      for k in range(NCHUNK)]
                for j, (pb, slot) in enumerate(ents):
                    for k in range(NCHUNK):
                        nc.tensor.matmul(
                            pa[k][:], w1t[:, 128 * slot:128 * slot + 128],
                            xt[pb][:, CH * k:CH * k + CH],
                            start=(j == 0), stop=(j == len(ents) - 1))
                for k in range(NCHUNK):
                    nc.scalar.copy(a3t[:, CH * k:CH * k + CH], pa[k][:])
                a3[t] = a3t

                for o in o_after_t[t]:
                    ents2 = plan["s2groups"][o]
                    ohi = sbo.tile([128, 2 * ROWS], BF16, tag="ohi", name="ohi")
                    for k in range(NCHUNK):
                        po = ps2.tile([128, CH], F32, tag="po", name="po")
                        for j, (t2, slot) in enumerate(ents2):
                            nc.tensor.matmul(
                                po[:], w2t[:, 128 * slot:128 * slot + 128],
                                a3[t2][:, CH * k:CH * k + CH],
                                start=(j == 0), stop=(j == len(ents2) - 1))
                        nc.scalar.copy(ohi[:, CH * k:CH * k + CH], po[:])
                        nc.vector.tensor_sub(
                            ohi[:, ROWS + CH * k:ROWS + CH * k + CH],
                            xt[o + 1][:, CH * k:CH * k + CH], po[:])
                    # one 8 KiB-line DMA per tile, alternating trigger
                    # engines; last tiles partition-split for tail latency
                    if o >= NO - 2:
                        nc.sync.dma_start(
                            out_d[128 * o:128 * o + 64, :], ohi[0:64, :])
                        nc.gpsimd.dma_start(
                            out_d[128 * o + 64:128 * o + 128, :], ohi[64:128, :])
                    elif o % 2 == 0:
                        nc.sync.dma_start(
                            out_d[128 * o:128 * o + 128, :], ohi[:])
                    else:
                        nc.gpsimd.dma_start(
                            out_d[128 * o:128 * o + 128, :], ohi[:])

    nc.compile()
    return nc


_CACHE = {}


def _get_plan_nc():
    if "pn" not in _CACHE:
        plan = _build_plan()
        nc = _build_program(plan)
        _CACHE["pn"] = (plan, nc)
    return _CACHE["pn"]


def _make_in_maps(plan, x):
    x = np.asarray(x)
    B, Fd, L_ = x.shape
    xs = np.ascontiguousarray(
        x.reshape(B * Fd, L_).T).astype(BF16_NP)   # sig layout [L, rows]
    in_maps = []
    for c in range(N_CORES):
        xbase = 2048 * c + X_OFF
        xloc = np.zeros((NPB * 128, ROWS), BF16_NP)
        lo_ = max(xbase, 0)
        hi_ = min(xbase + NPB * 128, L_)
        xloc[lo_ - xbase:hi_ - xbase] = xs[lo_:hi_]
        in_maps.append({"x": xloc, "w1": plan["w1"][c], "w2": plan["w2"][c]})
    return in_maps


def kernel(x):
    x = np.asarray(x)
    B, Fd, L_ = x.shape
    in_dtype = x.dtype
    plan, nc = _get_plan_nc()
    in_maps = _make_in_maps(plan, x)
    res = run_bass_kernel_spmd(nc, in_maps, list(range(N_CORES)))
    out = np.concatenate([np.asarray(r["out"]) for r in res.results], axis=0)
    low_sig, high_sig = out[:, :ROWS], out[:, ROWS:]
    low = np.ascontiguousarray(low_sig.T).astype(np.float32).reshape(B, Fd, L_)
    high = np.ascontiguousarray(high_sig.T).astype(np.float32).reshape(B, Fd, L_)
    return low.astype(in_dtype, copy=False), high.astype(in_dtype, copy=False)


# revision 22
# speedup vs baseline: 1.0401x; 1.0401x over previous
"""3-level db4 wavelet low/high split for (32, 64, 16384) fp32 on 8 TRN2 NeuronCores.

Math: the reference computes wavedec (3-level db4, symmetric padding) then two
waverecs: `low` (details zeroed) and `high` (approximation zeroed).  Wavelets
give perfect reconstruction, so low + high == x and only the lowpass path is
needed: low = G @ (H @ x_row) with H (2054 x 16384) the composite 3-level
lowpass analysis operator (symmetric extension folded in) and G (16384 x 2054)
the lowpass synthesis operator; high = x - low on-chip.

Layout/sharding: the signal axis L is sharded across the 8 cores (2048
positions each + 128-position halo).  The host uploads x TRANSPOSED into
"sig" layout [pos, rows] as bf16, so every on-device matmul streams the 2048
fused batch*feature rows as the moving operand and NO on-device transposes
are needed; outputs are written back in sig layout as bf16 and the host
re-transposes while unsharding.  Both operator stages are banded: per core
only 20 (stage 1) + 18 (stage 2) distinct 128x128 weight tiles are nonzero.

Device pipeline per core:
 - 18 x-tile DMAs [128 pos, 2048 rows] bf16 (4 KiB/partition lines)
 - stage 1 (PE, bf16): a3[t] += W1(t,pb)^T @ x[pb], accumulated in PSUM
   (4 banks of [128, 512] per a-tile), ACT-copied to SBUF bf16
 - stage 2 (PE, bf16): low[o] += W2(o,t)^T @ a3[t] in PSUM
 - ACT: low PSUM -> SBUF bf16; DVE: high = x - low(PSUM) -> bf16
 - per-o DMAs of low/high [128, 2048] bf16 back to HBM
"""

import numpy as np
import scipy.sparse as sp
import ml_dtypes

import concourse.bacc as bacc
import concourse.tile as tile
from concourse import mybir
from concourse.bass_utils import run_bass_kernel_spmd

F32 = mybir.dt.float32
BF16 = mybir.dt.bfloat16
BF16_NP = ml_dtypes.bfloat16

DEC_LO = np.array([-0.010597401785069032, 0.032883011666982945, 0.030841381835986965,
                   -0.18703481171888114, -0.02798376941698385, 0.6308807679295904,
                   0.7148465705525415, 0.23037781330885523], dtype=np.float64)
REC_LO = DEC_LO[::-1].copy()
F = 8
N_CORES = 8
L = 16384
ROWS = 2048          # fused B*F rows
S = L // N_CORES     # 2048 positions per core
NPB = 18             # local x tiles (128-position halo each side)
NT = 3               # local a3 tiles (384-value a window)
NO = 16              # local output tiles
A_OFF = -64          # a-window start, relative to 256*c
X_OFF = -128         # x-window start, relative to 2048*c
NCHUNK = 4           # 2048 rows in 4 PSUM-bank chunks of 512


def _symidx(n):
    idx = np.concatenate([np.arange(6, -1, -1), np.arange(n), np.arange(n - 1, n - 8, -1)])
    return idx[1:]


def _dwt_lo_mat(n):
    ext_idx = _symidx(n)
    lout = (n + 13 - F) // 2 + 1
    filt = DEC_LO[::-1]
    rows = np.repeat(np.arange(lout), F)
    cols = ext_idx[(2 * np.arange(lout)[:, None] + np.arange(F)[None, :]).ravel()]
    vals = np.tile(filt, lout)
    return sp.coo_matrix((vals, (rows, cols)), shape=(lout, n)).tocsr()


def _idwt_lo_mat(n):
    lout = 2 * n + 1 - F + 1
    filt = REC_LO[::-1]
    rows, cols, vals = [], [], []
    i = np.arange(lout)
    for k in range(F):
        pos = i + k
        m = (pos % 2 == 1)
        rows.append(i[m])
        cols.append((pos[m] - 1) // 2)
        vals.append(np.full(int(m.sum()), filt[k]))
    return sp.coo_matrix(
        (np.concatenate(vals), (np.concatenate(rows), np.concatenate(cols))),
        shape=(lout, n)).tocsr()


def _build_H_G(L, level=3):
    H = sp.identity(L, format="csr")
    lens = []
    n = L
    for _ in range(level):
        lens.append(n)
        D = _dwt_lo_mat(n)
        H = D @ H
        n = D.shape[0]
    G = sp.identity(n, format="csr")
    a_len = n
    for ln in lens[::-1]:
        d_len = (ln + F - 1) // 2
        if a_len == d_len + 1:
            G = sp.identity(a_len, format="csr")[:-1] @ G
            a_len -= 1
        U = _idwt_lo_mat(a_len)
        G = U @ G
        a_len = U.shape[0]
    return H, G


def _slice_pad(M, r0, r1, c0, c1):
    out = np.zeros((r1 - r0, c1 - c0), np.float32)
    rr0, rr1 = max(r0, 0), min(r1, M.shape[0])
    cc0, cc1 = max(c0, 0), min(c1, M.shape[1])
    if rr0 < rr1 and cc0 < cc1:
        out[rr0 - r0:rr1 - r0, cc0 - c0:cc1 - c0] = M[rr0:rr1, cc0:cc1]
    return out


def _build_plan():
    H, G = _build_H_G(L)
    HT = np.asarray(H.T.todense(), np.float32)   # [L, na]
    GT = np.asarray(G.T.todense(), np.float32)   # [na, L]

    s1_pairs, s2_pairs = set(), set()
    w1, w2 = {}, {}
    for c in range(N_CORES):
        xbase = 2048 * c + X_OFF
        abase = 256 * c + A_OFF
        for t in range(NT):
            a0 = abase + 128 * t
            for pb in range(NPB):
                p0 = xbase + 128 * pb
                tl = _slice_pad(HT, p0, p0 + 128, a0, a0 + 128)
                if np.any(tl):
                    s1_pairs.add((t, pb))
                    w1[(c, t, pb)] = tl
        for o in range(NO):
            i0 = 2048 * c + 128 * o
            for t in range(NT):
                a0 = abase + 128 * t
                tl = _slice_pad(GT, a0, a0 + 128, i0, i0 + 128)
                if np.any(tl):
                    s2_pairs.add((o, t))
                    w2[(c, o, t)] = tl
    s1_pairs = sorted(s1_pairs)
    s2_pairs = sorted(s2_pairs)

    # weight blobs per core, one 128x128 slot per pair (zeros where the core
    # has no tile); stage-1 slots keyed by (t, pb), stage-2 by (o, t)
    w1_blob = np.zeros((N_CORES, 128, 128 * len(s1_pairs)), BF16_NP)
    w2_blob = np.zeros((N_CORES, 128, 128 * len(s2_pairs)), BF16_NP)
    for c in range(N_CORES):
        for i, (t, pb) in enumerate(s1_pairs):
            tl = w1.get((c, t, pb))
            if tl is not None:
                w1_blob[c, :, 128 * i:128 * i + 128] = tl.astype(BF16_NP)
        for i, (o, t) in enumerate(s2_pairs):
            tl = w2.get((c, o, t))
            if tl is not None:
                w2_blob[c, :, 128 * i:128 * i + 128] = tl.astype(BF16_NP)

    s1groups = [[] for _ in range(NT)]          # t -> [(pb, slot)]
    for i, (t, pb) in enumerate(s1_pairs):
        s1groups[t].append((pb, i))
    s2groups = [[] for _ in range(NO)]          # o -> [(t, slot)]
    for i, (o, t) in enumerate(s2_pairs):
        s2groups[o].append((t, i))
    return dict(w1=w1_blob, w2=w2_blob, s1groups=s1groups, s2groups=s2groups,
                n1=len(s1_pairs), n2=len(s2_pairs))


def _build_program(plan):
    nc = bacc.Bacc("TRN2", target_bir_lowering=False, debug=False)
    x_d = nc.dram_tensor("x", [NPB * 128, ROWS], BF16, kind="ExternalInput").ap()
    w1_d = nc.dram_tensor("w1", [128, 128 * plan["n1"]], BF16, kind="ExternalInput").ap()
    w2_d = nc.dram_tensor("w2", [128, 128 * plan["n2"]], BF16, kind="ExternalInput").ap()
    low_d = nc.dram_tensor("low", [S, ROWS], BF16, kind="ExternalOutput").ap()
    high_d = nc.dram_tensor("high", [S, ROWS], BF16, kind="ExternalOutput").ap()

    CH = ROWS // NCHUNK  # 512

    with tile.TileContext(nc) as tc:
        with tc.tile_pool(name="sbw", bufs=1) as sbw, \
             tc.tile_pool(name="sbx", bufs=1) as sbx, \
             tc.tile_pool(name="sba3", bufs=1) as sba3, \
             tc.tile_pool(name="sbo", bufs=6) as sbo, \
             tc.tile_pool(name="psa", bufs=1, space="PSUM") as psa, \
             tc.tile_pool(name="ps2", bufs=4, space="PSUM") as ps2:

            # weights via scalar/vector triggers (idle at program head); the
            # first w1 quarter (earliest-needed slots) partition-split for
            # low arrival latency
            w1t = sbw.tile([128, 128 * plan["n1"]], BF16, tag="w1t")
            q = (plan["n1"] * 128) // 4
            nc.scalar.dma_start(w1t[0:64, 0:q], w1_d[0:64, 0:q])
            nc.scalar.dma_start(w1t[64:128, 0:q], w1_d[64:128, 0:q])
            nc.scalar.dma_start(w1t[:, q:2 * q], w1_d[:, q:2 * q])
            nc.scalar.dma_start(w1t[:, 2 * q:3 * q], w1_d[:, 2 * q:3 * q])
            nc.scalar.dma_start(w1t[:, 3 * q:4 * q], w1_d[:, 3 * q:4 * q])
            w2t = sbw.tile([128, 128 * plan["n2"]], BF16, tag="w2t")
            q2 = (plan["n2"] * 128) // 2
            nc.scalar.dma_start(w2t[:, 0:q2], w2_d[:, 0:q2])
            nc.scalar.dma_start(w2t[:, q2:2 * q2], w2_d[:, q2:2 * q2])

            # x tiles: every tile partition-split across two trigger engines
            # (sync + gpsimd) so arrivals stay dense for the PE.  The halo
            # tiles only need a sliver of real data (stage-2 never reads the
            # a3 values their dead positions feed) — load the sliver and
            # memset the rest so no NaN garbage enters the matmuls.
            xt = [sbx.tile([128, ROWS], BF16, tag=f"x{pb}", name=f"x{pb}")
                  for pb in range(NPB)]
            # composite H rows span x in [8j-42, 8j+7], so the halo tiles
            # only need their inner 64 partitions of real data
            nc.vector.memset(xt[0][0:64, :], 0)
            nc.vector.memset(xt[NPB - 1][64:128, :], 0)
            nc.sync.dma_start(xt[0][64:128, :], x_d[64:128, :])
            p17 = 128 * (NPB - 1)
            nc.gpsimd.dma_start(xt[NPB - 1][0:64, :], x_d[p17:p17 + 64, :])
            for pb in range(1, NPB - 1):
                nc.sync.dma_start(xt[pb][0:64, :], x_d[128 * pb:128 * pb + 64, :])
                nc.gpsimd.dma_start(
                    xt[pb][64:128, :], x_d[128 * pb + 64:128 * pb + 128, :])

            # output tiles o grouped by the last a3 tile they need
            o_after_t = [[] for _ in range(NT)]
            for o in range(NO):
                o_after_t[max(t for t, _ in plan["s2groups"][o])].append(o)

            a3 = [None] * NT
            for t in range(NT):
                ents = plan["s1groups"][t]
                a3t = sba3.tile([128, ROWS], BF16, tag=f"a3_{t}", name=f"a3_{t}")
                # pair-outer so each x tile is consumed the moment it lands;
                # the four chunk accumulation groups interleave in PSUM
                pa = [psa.tile([128, CH], F32, tag=f"pa{k}", name=f"pa{k}")
                      for k in range(NCHUNK)]
                for j, (pb, slot) in enumerate(ents):
                    for k in range(NCHUNK):
                        nc.tensor.matmul(
                            pa[k][:], w1t[:, 128 * slot:128 * slot + 128],
                            xt[pb][:, CH * k:CH * k + CH],
                            start=(j == 0), stop=(j == len(ents) - 1))
                for k in range(NCHUNK):
                    nc.scalar.copy(a3t[:, CH * k:CH * k + CH], pa[k][:])
                a3[t] = a3t

                for o in o_after_t[t]:
                    ents2 = plan["s2groups"][o]
                    lo = sbo.tile([128, ROWS], BF16, tag="lo", name="lo")
                    hi = sbo.tile([128, ROWS], BF16, tag="hi", name="hi")
                    for k in range(NCHUNK):
                        po = ps2.tile([128, CH], F32, tag="po", name="po")
                        for j, (t2, slot) in enumerate(ents2):
                            nc.tensor.matmul(
                                po[:], w2t[:, 128 * slot:128 * slot + 128],
                                a3[t2][:, CH * k:CH * k + CH],
                                start=(j == 0), stop=(j == len(ents2) - 1))
                        nc.scalar.copy(lo[:, CH * k:CH * k + CH], po[:])
                        nc.vector.tensor_sub(
                            hi[:, CH * k:CH * k + CH],
                            xt[o + 1][:, CH * k:CH * k + CH], po[:])
                    # full-width DMAs keep 4 KiB partition lines (smaller
                    # lines are descriptor-rate-bound); last tiles get
                    # partition-split halves for tail latency
                    if o >= NO - 2:
                        for p0, p1 in ((0, 64), (64, 128)):
                            nc.sync.dma_start(
                                low_d[128 * o + p0:128 * o + p1, :], lo[p0:p1, :])
                            nc.gpsimd.dma_start(
                                high_d[128 * o + p0:128 * o + p1, :], hi[p0:p1, :])
                    else:
                        nc.sync.dma_start(low_d[128 * o:128 * o + 128, :], lo[:])
                        nc.gpsimd.dma_start(high_d[128 * o:128 * o + 128, :], hi[:])

    nc.compile()
    return nc


_CACHE = {}


def _get_plan_nc():
    if "pn" not in _CACHE:
        plan = _build_plan()
        nc = _build_program(plan)
        _CACHE["pn"] = (plan, nc)
    return _CACHE["pn"]


def _make_in_maps(plan, x):
    x = np.asarray(x)
    B, Fd, L_ = x.shape
    xs = np.ascontiguousarray(
        x.reshape(B * Fd, L_).T).astype(BF16_NP)   # sig layout [L, rows]
    in_maps = []
    for c in range(N_CORES):
        xbase = 2048 * c + X_OFF
        xloc = np.zeros((NPB * 128, ROWS), BF16_NP)
        lo_ = max(xbase, 0)
        hi_ = min(xbase + NPB * 128, L_)
        xloc[lo_ - xbase:hi_ - xbase] = xs[lo_:hi_]
        in_maps.append({"x": xloc, "w1": plan["w1"][c], "w2": plan["w2"][c]})
    return in_maps


def kernel(x):
    x = np.asarray(x)
    B, Fd, L_ = x.shape
    in_dtype = x.dtype
    plan, nc = _get_plan_nc()
    in_maps = _make_in_maps(plan, x)
    res = run_bass_kernel_spmd(nc, in_maps, list(range(N_CORES)))
    low_sig = np.concatenate([np.asarray(r["low"]) for r in res.results], axis=0)
    high_sig = np.concatenate([np.asarray(r["high"]) for r in res.results], axis=0)
    low = np.ascontiguousarray(low_sig.T).astype(np.float32).reshape(B, Fd, L_)
    high = np.ascontiguousarray(high_sig.T).astype(np.float32).reshape(B, Fd, L_)
    return low.astype(in_dtype, copy=False), high.astype(in_dtype, copy=False)


# revision 23
# speedup vs baseline: 1.0695x; 1.0283x over previous
"""3-level db4 wavelet low/high split for (32, 64, 16384) fp32 on 8 TRN2 NeuronCores.

Math: the reference computes wavedec (3-level db4, symmetric padding) then two
waverecs: `low` (details zeroed) and `high` (approximation zeroed).  Wavelets
give perfect reconstruction, so low + high == x and only the lowpass path is
needed: low = G @ (H @ x_row) with H (2054 x 16384) the composite 3-level
lowpass analysis operator (symmetric extension folded in) and G (16384 x 2054)
the lowpass synthesis operator; high = x - low on-chip.

Layout/sharding: the signal axis L is sharded across the 8 cores (2048
positions each + 128-position halo).  The host uploads x TRANSPOSED into
"sig" layout [pos, rows] as bf16, so every on-device matmul streams the 2048
fused batch*feature rows as the moving operand and NO on-device transposes
are needed; outputs are written back in sig layout as bf16 and the host
re-transposes while unsharding.  Both operator stages are banded: per core
only 20 (stage 1) + 18 (stage 2) distinct 128x128 weight tiles are nonzero.

Device pipeline per core:
 - 18 x-tile DMAs [128 pos, 2048 rows] bf16 (4 KiB/partition lines)
 - stage 1 (PE, bf16): a3[t] += W1(t,pb)^T @ x[pb], accumulated in PSUM
   (4 banks of [128, 512] per a-tile), ACT-copied to SBUF bf16
 - stage 2 (PE, bf16): low[o] += W2(o,t)^T @ a3[t] in PSUM
 - ACT: low PSUM -> SBUF bf16; DVE: high = x - low(PSUM) -> bf16
 - per-o DMAs of low/high [128, 2048] bf16 back to HBM
"""

import numpy as np
import scipy.sparse as sp
import ml_dtypes

import concourse.bacc as bacc
import concourse.tile as tile
from concourse import mybir
from concourse.bass_utils import run_bass_kernel_spmd

F32 = mybir.dt.float32
BF16 = mybir.dt.bfloat16
BF16_NP = ml_dtypes.bfloat16

DEC_LO = np.array([-0.010597401785069032, 0.032883011666982945, 0.030841381835986965,
                   -0.18703481171888114, -0.02798376941698385, 0.6308807679295904,
                   0.7148465705525415, 0.23037781330885523], dtype=np.float64)
REC_LO = DEC_LO[::-1].copy()
F = 8
N_CORES = 8
L = 16384
ROWS = 2048          # fused B*F rows
S = L // N_CORES     # 2048 positions per core
NPB = 18             # local x tiles (128-position halo each side)
NT = 3               # local a3 tiles (384-value a window)
NO = 16              # local output tiles
A_OFF = -64          # a-window start, relative to 256*c
X_OFF = -128         # x-window start, relative to 2048*c
NCHUNK = 4           # 2048 rows in 4 PSUM-bank chunks of 512


def _symidx(n):
    idx = np.concatenate([np.arange(6, -1, -1), np.arange(n), np.arange(n - 1, n - 8, -1)])
    return idx[1:]


def _dwt_lo_mat(n):
    ext_idx = _symidx(n)
    lout = (n + 13 - F) // 2 + 1
    filt = DEC_LO[::-1]
    rows = np.repeat(np.arange(lout), F)
    cols = ext_idx[(2 * np.arange(lout)[:, None] + np.arange(F)[None, :]).ravel()]
    vals = np.tile(filt, lout)
    return sp.coo_matrix((vals, (rows, cols)), shape=(lout, n)).tocsr()


def _idwt_lo_mat(n):
    lout = 2 * n + 1 - F + 1
    filt = REC_LO[::-1]
    rows, cols, vals = [], [], []
    i = np.arange(lout)
    for k in range(F):
        pos = i + k
        m = (pos % 2 == 1)
        rows.append(i[m])
        cols.append((pos[m] - 1) // 2)
        vals.append(np.full(int(m.sum()), filt[k]))
    return sp.coo_matrix(
        (np.concatenate(vals), (np.concatenate(rows), np.concatenate(cols))),
        shape=(lout, n)).tocsr()


def _build_H_G(L, level=3):
    H = sp.identity(L, format="csr")
    lens = []
    n = L
    for _ in range(level):
        lens.append(n)
        D = _dwt_lo_mat(n)
        H = D @ H
        n = D.shape[0]
    G = sp.identity(n, format="csr")
    a_len = n
    for ln in lens[::-1]:
        d_len = (ln + F - 1) // 2
        if a_len == d_len + 1:
            G = sp.identity(a_len, format="csr")[:-1] @ G
            a_len -= 1
        U = _idwt_lo_mat(a_len)
        G = U @ G
        a_len = U.shape[0]
    return H, G


def _slice_pad(M, r0, r1, c0, c1):
    out = np.zeros((r1 - r0, c1 - c0), np.float32)
    rr0, rr1 = max(r0, 0), min(r1, M.shape[0])
    cc0, cc1 = max(c0, 0), min(c1, M.shape[1])
    if rr0 < rr1 and cc0 < cc1:
        out[rr0 - r0:rr1 - r0, cc0 - c0:cc1 - c0] = M[rr0:rr1, cc0:cc1]
    return out


def _build_plan():
    H, G = _build_H_G(L)
    HT = np.asarray(H.T.todense(), np.float32)   # [L, na]
    GT = np.asarray(G.T.todense(), np.float32)   # [na, L]

    s1_pairs, s2_pairs = set(), set()
    w1, w2 = {}, {}
    for c in range(N_CORES):
        xbase = 2048 * c + X_OFF
        abase = 256 * c + A_OFF
        for t in range(NT):
            a0 = abase + 128 * t
            for pb in range(NPB):
                p0 = xbase + 128 * pb
                tl = _slice_pad(HT, p0, p0 + 128, a0, a0 + 128)
                if np.any(tl):
                    s1_pairs.add((t, pb))
                    w1[(c, t, pb)] = tl
        for o in range(NO):
            i0 = 2048 * c + 128 * o
            for t in range(NT):
                a0 = abase + 128 * t
                tl = _slice_pad(GT, a0, a0 + 128, i0, i0 + 128)
                if np.any(tl):
                    s2_pairs.add((o, t))
                    w2[(c, o, t)] = tl
    s1_pairs = sorted(s1_pairs)
    s2_pairs = sorted(s2_pairs)

    # weight blobs per core, one 128x128 slot per pair (zeros where the core
    # has no tile); stage-1 slots keyed by (t, pb), stage-2 by (o, t)
    w1_blob = np.zeros((N_CORES, 128, 128 * len(s1_pairs)), BF16_NP)
    w2_blob = np.zeros((N_CORES, 128, 128 * len(s2_pairs)), BF16_NP)
    for c in range(N_CORES):
        for i, (t, pb) in enumerate(s1_pairs):
            tl = w1.get((c, t, pb))
            if tl is not None:
                w1_blob[c, :, 128 * i:128 * i + 128] = tl.astype(BF16_NP)
        for i, (o, t) in enumerate(s2_pairs):
            tl = w2.get((c, o, t))
            if tl is not None:
                w2_blob[c, :, 128 * i:128 * i + 128] = tl.astype(BF16_NP)

    s1groups = [[] for _ in range(NT)]          # t -> [(pb, slot)]
    for i, (t, pb) in enumerate(s1_pairs):
        s1groups[t].append((pb, i))
    s2groups = [[] for _ in range(NO)]          # o -> [(t, slot)]
    for i, (o, t) in enumerate(s2_pairs):
        s2groups[o].append((t, i))
    return dict(w1=w1_blob, w2=w2_blob, s1groups=s1groups, s2groups=s2groups,
                n1=len(s1_pairs), n2=len(s2_pairs))


def _build_program(plan):
    nc = bacc.Bacc("TRN2", target_bir_lowering=False, debug=False)
    x_d = nc.dram_tensor("x", [NPB * 128, ROWS], BF16, kind="ExternalInput").ap()
    w1_d = nc.dram_tensor("w1", [128, 128 * plan["n1"]], BF16, kind="ExternalInput").ap()
    w2_d = nc.dram_tensor("w2", [128, 128 * plan["n2"]], BF16, kind="ExternalInput").ap()
    low_d = nc.dram_tensor("low", [S, ROWS], BF16, kind="ExternalOutput").ap()
    high_d = nc.dram_tensor("high", [S, ROWS], BF16, kind="ExternalOutput").ap()

    CH = ROWS // NCHUNK  # 512

    with tile.TileContext(nc) as tc:
        with tc.tile_pool(name="sbw", bufs=1) as sbw, \
             tc.tile_pool(name="sbx", bufs=1) as sbx, \
             tc.tile_pool(name="sba3", bufs=1) as sba3, \
             tc.tile_pool(name="sbo", bufs=6) as sbo, \
             tc.tile_pool(name="psa", bufs=1, space="PSUM") as psa, \
             tc.tile_pool(name="ps2", bufs=4, space="PSUM") as ps2:

            # weights via scalar/vector triggers (idle at program head); the
            # first w1 quarter (earliest-needed slots) partition-split for
            # low arrival latency
            w1t = sbw.tile([128, 128 * plan["n1"]], BF16, tag="w1t")
            q = (plan["n1"] * 128) // 4
            nc.scalar.dma_start(w1t[0:64, 0:q], w1_d[0:64, 0:q])
            nc.scalar.dma_start(w1t[64:128, 0:q], w1_d[64:128, 0:q])
            w2t = sbw.tile([128, 128 * plan["n2"]], BF16, tag="w2t")
            q2 = (plan["n2"] * 128) // 2
            # all weight slices partition-split: 64-descriptor chains halve
            # arrival latency so neither t=1 stage-1 nor the first o-tiles
            # stall on weights
            for p0, p1 in ((0, 64), (64, 128)):
                nc.scalar.dma_start(w2t[p0:p1, 0:q2], w2_d[p0:p1, 0:q2])
            for i in range(1, 4):
                for p0, p1 in ((0, 64), (64, 128)):
                    nc.scalar.dma_start(
                        w1t[p0:p1, i * q:(i + 1) * q], w1_d[p0:p1, i * q:(i + 1) * q])
            for p0, p1 in ((0, 64), (64, 128)):
                nc.scalar.dma_start(w2t[p0:p1, q2:2 * q2], w2_d[p0:p1, q2:2 * q2])

            # x tiles: every tile partition-split across two trigger engines
            # (sync + gpsimd) so arrivals stay dense for the PE.  The halo
            # tiles only need a sliver of real data (stage-2 never reads the
            # a3 values their dead positions feed) — load the sliver and
            # memset the rest so no NaN garbage enters the matmuls.
            xt = [sbx.tile([128, ROWS], BF16, tag=f"x{pb}", name=f"x{pb}")
                  for pb in range(NPB)]
            # composite H rows span x in [8j-42, 8j+7], so the halo tiles
            # only need their inner 64 partitions of real data
            nc.vector.memset(xt[0][0:64, :], 0)
            nc.vector.memset(xt[NPB - 1][64:128, :], 0)
            nc.sync.dma_start(xt[0][64:128, :], x_d[64:128, :])
            p17 = 128 * (NPB - 1)
            nc.gpsimd.dma_start(xt[NPB - 1][0:64, :], x_d[p17:p17 + 64, :])
            for pb in range(1, NPB - 1):
                nc.sync.dma_start(xt[pb][0:64, :], x_d[128 * pb:128 * pb + 64, :])
                nc.gpsimd.dma_start(
                    xt[pb][64:128, :], x_d[128 * pb + 64:128 * pb + 128, :])

            # output tiles o grouped by the last a3 tile they need
            o_after_t = [[] for _ in range(NT)]
            for o in range(NO):
                o_after_t[max(t for t, _ in plan["s2groups"][o])].append(o)

            a3 = [None] * NT
            for t in range(NT):
                ents = plan["s1groups"][t]
                a3t = sba3.tile([128, ROWS], BF16, tag=f"a3_{t}", name=f"a3_{t}")
                # pair-outer so each x tile is consumed the moment it lands;
                # the four chunk accumulation groups interleave in PSUM
                pa = [psa.tile([128, CH], F32, tag=f"pa{k}", name=f"pa{k}")
                      for k in range(NCHUNK)]
                for j, (pb, slot) in enumerate(ents):
                    for k in range(NCHUNK):
                        nc.tensor.matmul(
                            pa[k][:], w1t[:, 128 * slot:128 * slot + 128],
                            xt[pb][:, CH * k:CH * k + CH],
                            start=(j == 0), stop=(j == len(ents) - 1))
                for k in range(NCHUNK):
                    nc.scalar.copy(a3t[:, CH * k:CH * k + CH], pa[k][:])
                a3[t] = a3t

                for o in o_after_t[t]:
                    ents2 = plan["s2groups"][o]
                    lo = sbo.tile([128, ROWS], BF16, tag="lo", name="lo")
                    hi = sbo.tile([128, ROWS], BF16, tag="hi", name="hi")
                    for k in range(NCHUNK):
                        po = ps2.tile([128, CH], F32, tag="po", name="po")
                        for j, (t2, slot) in enumerate(ents2):
                            nc.tensor.matmul(
                                po[:], w2t[:, 128 * slot:128 * slot + 128],
                                a3[t2][:, CH * k:CH * k + CH],
                                start=(j == 0), stop=(j == len(ents2) - 1))
                        nc.scalar.copy(lo[:, CH * k:CH * k + CH], po[:])
                        nc.vector.tensor_sub(
                            hi[:, CH * k:CH * k + CH],
                            xt[o + 1][:, CH * k:CH * k + CH], po[:])
                    # full-width DMAs keep 4 KiB partition lines (smaller
                    # lines are descriptor-rate-bound); last tiles get
                    # partition-split halves for tail latency
                    if o >= NO - 2:
                        for p0, p1 in ((0, 64), (64, 128)):
                            nc.sync.dma_start(
                                low_d[128 * o + p0:128 * o + p1, :], lo[p0:p1, :])
                            nc.gpsimd.dma_start(
                                high_d[128 * o + p0:128 * o + p1, :], hi[p0:p1, :])
                    else:
                        nc.sync.dma_start(low_d[128 * o:128 * o + 128, :], lo[:])
                        nc.gpsimd.dma_start(high_d[128 * o:128 * o + 128, :], hi[:])

    nc.compile()
    return nc


_CACHE = {}


def _get_plan_nc():
    if "pn" not in _CACHE:
        plan = _build_plan()
        nc = _build_program(plan)
        _CACHE["pn"] = (plan, nc)
    return _CACHE["pn"]


def _make_in_maps(plan, x):
    x = np.asarray(x)
    B, Fd, L_ = x.shape
    xs = np.ascontiguousarray(
        x.reshape(B * Fd, L_).T).astype(BF16_NP)   # sig layout [L, rows]
    in_maps = []
    for c in range(N_CORES):
        xbase = 2048 * c + X_OFF
        xloc = np.zeros((NPB * 128, ROWS), BF16_NP)
        lo_ = max(xbase, 0)
        hi_ = min(xbase + NPB * 128, L_)
        xloc[lo_ - xbase:hi_ - xbase] = xs[lo_:hi_]
        in_maps.append({"x": xloc, "w1": plan["w1"][c], "w2": plan["w2"][c]})
    return in_maps


def kernel(x):
    x = np.asarray(x)
    B, Fd, L_ = x.shape
    in_dtype = x.dtype
    plan, nc = _get_plan_nc()
    in_maps = _make_in_maps(plan, x)
    res = run_bass_kernel_spmd(nc, in_maps, list(range(N_CORES)))
    low_sig = np.concatenate([np.asarray(r["low"]) for r in res.results], axis=0)
    high_sig = np.concatenate([np.asarray(r["high"]) for r in res.results], axis=0)
    low = np.ascontiguousarray(low_sig.T).astype(np.float32).reshape(B, Fd, L_)
    high = np.ascontiguousarray(high_sig.T).astype(np.float32).reshape(B, Fd, L_)
    return low.astype(in_dtype, copy=False), high.astype(in_dtype, copy=False)
